# revision 1
# baseline (speedup 1.0000x reference)
"""Multi-head attention (B=4, S=2048, d_model=1024, 16 heads x 64) on 8 trn2
NeuronCores — v2.

Sharding: core c -> (batch b = c//2, head-group g = c%2); 8 heads per core.
Host sums the two partial output projections per batch and adds bo.

Dataflow (all fp16 operands, f32 PSUM):
  proj:   qhT/khT [128 = pair-dims, 2048] fp16 (dims on partitions)
          vh_all  [128 = k-pos, h, kt, 65] fp16 (ones col 64 = denominator)
  scores: per (head, qhalf, kt): out[k 128, q 1024] = khT_h^T-slice @ qhT_h
  exp:    E**s with E = e^(1/8), split across ACT (native Exp), DVE and Pool
          (tensor_tensor pow) to break the single-engine activation floor
  AV:     flipped — out[q 128, 65] = pt_slice^T @ vh (16 kt accum in PSUM),
          8 q-subtiles per [128, 1024] pt tile; 2.4x fewer PE cycles than
          the d-on-partitions orientation
  norm:   reciprocal of col 64 + per-partition scale (q is on partitions)
  attnT:  DMA-transpose [128 q, 128 pair-dims] -> [128, 128] into attnT[c]
  oproj:  out[seq 128, 1024] = attnT_c-slice @ wo_c, 4-chunk accum
"""

import numpy as np

import concourse.bass as bass
import concourse.bacc as bacc
import concourse.mybir as mybir
import concourse.tile as tile
from concourse import bass_utils
from concourse.alu_op_type import AluOpType

F32 = mybir.dt.float32
F16 = mybir.dt.float16

B, S, DM = 4, 2048, 1024
HPC = 8          # heads per core
DK = DV = 64
NP = HPC // 2    # head pairs per core = 4
KT = S // 128    # 16 k-tiles
KC = DM // 128   # 8 contraction chunks
EXP8 = float(np.exp(0.125))
ACT_LAG = 3
POOL_LAG = 7

# exp engine split: ACT runs native Exp straight from PSUM; a ~40% share
# of tiles goes DVE-copy (PSUM->SBUF) + Pool pow (E**s, SBUF->SBUF) since
# walrus rejects pow on DVE and GPSIMD cannot read PSUM.
def exp_on_pool(i):
    return i % 7 in (1, 3, 5)


def build_nc():
    nc = bacc.Bacc("TRN2", target_bir_lowering=False)

    qT = nc.dram_tensor("qT", [4, KC, 128, 512], F16, kind="ExternalInput")
    kT = nc.dram_tensor("kT", [4, KC, 128, 512], F16, kind="ExternalInput")
    vT = nc.dram_tensor("vT", [4, KC, 128, 512], F16, kind="ExternalInput")
    wq = nc.dram_tensor("wq", [128, KC, 512], F16, kind="ExternalInput")
    wk = nc.dram_tensor("wk", [128, KC, 512], F16, kind="ExternalInput")
    wv = nc.dram_tensor("wv", [128, KC, 512], F16, kind="ExternalInput")
    wo = nc.dram_tensor("wo", [128, NP, 1024], F16, kind="ExternalInput")
    bq = nc.dram_tensor("bq", [128, NP], F32, kind="ExternalInput")
    bk = nc.dram_tensor("bk", [128, NP], F32, kind="ExternalInput")
    bv = nc.dram_tensor("bv", [512], F32, kind="ExternalInput")
    out = nc.dram_tensor("out", [S, 1024], F32, kind="ExternalOutput")

    def dram_ap(t, offset, ap):
        base = t[:]
        return bass.AP(tensor=base.tensor, offset=base.offset + offset, ap=ap)

    # [qq] chunk of qT as [128, kc, 512] (partition-major view of [kc,128,512])
    def x_chunk_ap(t, qq):
        return dram_ap(t, qq * KC * 128 * 512,
                       [[512, 128], [128 * 512, KC], [1, 512]])

    with tile.TileContext(nc) as tc:
        from contextlib import ExitStack

        with ExitStack() as est:
            # ---------------- persistent SBUF pools ----------------
            w_pool = est.enter_context(tc.tile_pool(name="wp", bufs=1))
            bias_pool = est.enter_context(tc.tile_pool(name="bias", bufs=1))
            vh_pool = est.enter_context(tc.tile_pool(name="vhp", bufs=1))
            qk_pool = est.enter_context(tc.tile_pool(name="qkt", bufs=1))
            at_pool = est.enter_context(tc.tile_pool(name="atp", bufs=1))
            e_pool = est.enter_context(tc.tile_pool(name="ep", bufs=1))

            wq_sb = w_pool.tile([128, KC, 512], F16, name="wq_sb")
            wk_sb = w_pool.tile([128, KC, 512], F16, name="wk_sb")
            wv_sb = w_pool.tile([128, KC, 512], F16, name="wv_sb")
            wo_sb = w_pool.tile([128, NP, 1024], F16, name="wo_sb")
            bq_sb = bias_pool.tile([128, NP], F32, name="bq_sb")
            bk_sb = bias_pool.tile([128, NP], F32, name="bk_sb")
            bv_bc = bias_pool.tile([128, 512], F32, name="bv_bc")
            e_sb = e_pool.tile([128, 1024], F32, name="e_sb")
            nc.vector.memset(e_sb, EXP8)

            # vh_all[k, h, t, 0:64] = vh rows t*128..t*128+128 for head h
            # vh_all[k, h, t, 64] = 1.0 (denominator column)
            # col 64 = ones (softmax denominator), col 65 = zero pad so
            # the fp16 moving operand has an even element count
            vh_all = vh_pool.tile([128, HPC, KT, DV + 2], F16, name="vh_all")
            nc.vector.memset(vh_all[:, :, :, DV:DV + 1], 1.0)
            nc.vector.memset(vh_all[:, :, :, DV + 1:DV + 2], 0.0)

            # weight/bias loads (first so phase V can start early)
            nc.sync.dma_start(out=wv_sb, in_=wv[:, :, :])
            bv_ap = bv[:]
            nc.sync.dma_start(
                out=bv_bc,
                in_=bass.AP(tensor=bv_ap.tensor, offset=bv_ap.offset,
                            ap=[[0, 128]] + list(bv_ap.ap)),
            )

            qhTs, khTs, attnTs = {}, {}, {}
            for p in range(NP):
                qhTs[p] = qk_pool.tile([128, S], F16, name=f"qhT{p}")
                khTs[p] = qk_pool.tile([128, S], F16, name=f"khT{p}")
                attnTs[p] = at_pool.tile([128, S], F16, name=f"attnT{p}")

            # ---------------- phase V: v projection ----------------
            x_pool = est.enter_context(tc.tile_pool(name="xch", bufs=1))
            with tc.tile_pool(name="psV", bufs=8, space="PSUM") as psV:
                for sq in range(4):
                    vch = x_pool.tile([128, KC, 512], F16, name="vch",
                                      tag="xv", bufs=2)
                    nc.sync.dma_start(out=vch, in_=x_chunk_ap(vT, sq))
                    if sq == 0:
                        # phase-A weights ride behind the first x chunk
                        nc.sync.dma_start(out=wq_sb, in_=wq[:, :, :])
                        nc.sync.dma_start(out=wk_sb, in_=wk[:, :, :])
                        nc.sync.dma_start(out=wo_sb, in_=wo[:, :, :])
                        nc.sync.dma_start(out=bq_sb, in_=bq[:, :])
                        nc.sync.dma_start(out=bk_sb, in_=bk[:, :])
                    pss = [psV.tile([128, 512], F32, name=f"psv{j}",
                                    tag="psv") for j in range(4)]
                    for kc in range(KC):
                        for j in range(4):
                            nc.tensor.matmul(
                                pss[j],
                                lhsT=vch[:, kc, j * 128:(j + 1) * 128],
                                rhs=wv_sb[:, kc, :],
                                start=(kc == 0), stop=(kc == KC - 1))
                    for j in range(4):
                        st = sq * 4 + j
                        nc.vector.tensor_add(
                            vh_all[:, :, st, 0:DV],
                            pss[j].rearrange("p (h d) -> p h d", h=HPC),
                            bv_bc.rearrange("p (h d) -> p h d", h=HPC))

            # ---------------- phase A: q/k projections ----------------
            with tc.tile_pool(name="psA", bufs=1, space="PSUM") as psA:
                qchs = {}
                for qq in range(4):
                    qch = x_pool.tile([128, KC, 512], F16, name="qch",
                                      tag="xq", bufs=2)
                    qchs[qq] = qch
                    nc.sync.dma_start(out=qch, in_=x_chunk_ap(qT, qq))
                    kch = x_pool.tile([128, KC, 512], F16, name="kch",
                                      tag="xk", bufs=2)
                    nc.sync.dma_start(out=kch, in_=x_chunk_ap(kT, qq))
                    psq = [psA.tile([128, 512], F32, name=f"psq{p}",
                                    tag=f"paq{p}") for p in range(NP)]
                    psk = [psA.tile([128, 512], F32, name=f"psk{p}",
                                    tag=f"pak{p}") for p in range(NP)]
                    for kc in range(KC):
                        for p in range(NP):
                            if qq < 2:
                                nc.tensor.matmul(
                                    psq[p],
                                    lhsT=wq_sb[:, kc, p * 128:(p + 1) * 128],
                                    rhs=qch[:, kc, :],
                                    start=(kc == 0), stop=(kc == KC - 1))
                            nc.tensor.matmul(
                                psk[p],
                                lhsT=wk_sb[:, kc, p * 128:(p + 1) * 128],
                                rhs=kch[:, kc, :],
                                start=(kc == 0), stop=(kc == KC - 1))
                    sl = slice(qq * 512, (qq + 1) * 512)
                    for p in range(NP):
                        if qq < 2:
                            nc.vector.tensor_scalar_add(qhTs[p][:, sl],
                                                        psq[p],
                                                        bq_sb[:, p:p + 1])
                        nc.vector.tensor_scalar_add(khTs[p][:, sl], psk[p],
                                                    bk_sb[:, p:p + 1])

            # ---------------- phase B: attention ----------------
            pt_pool = est.enter_context(tc.tile_pool(name="ptp", bufs=12))
            scsb_pool = est.enter_context(tc.tile_pool(name="scsb", bufs=5))
            avsb_pool = est.enter_context(tc.tile_pool(name="avsb", bufs=2))
            rc_pool = est.enter_context(tc.tile_pool(name="rcp", bufs=2))
            osb_pool = est.enter_context(tc.tile_pool(name="osb", bufs=2))

            with tc.tile_pool(name="psS", bufs=3, space="PSUM") as psS, \
                 tc.tile_pool(name="psAV", bufs=1, space="PSUM") as psAV:

                def emit_oproj(t):
                    tsl = slice(t * 128, (t + 1) * 128)
                    pf = psS.tile([128, 1024], F32, name="pf", tag="sc")
                    for c in range(NP):
                        for half in range(2):
                            nc.tensor.matmul(
                                pf[:, half * 512:(half + 1) * 512],
                                lhsT=attnTs[c][:, tsl],
                                rhs=wo_sb[:, c, half * 512:(half + 1) * 512],
                                start=(c == 0), stop=(c == NP - 1))
                    fs = osb_pool.tile([128, 1024], F32, name="fs", tag="fs")
                    if t % 2 == 0:
                        nc.scalar.copy(fs, pf)
                    else:
                        nc.vector.tensor_copy(fs, pf)
                    nc.sync.dma_start(out=out[tsl, :], in_=fs)

                def emit_qproj(qq):
                    sl = slice(qq * 512, (qq + 1) * 512)
                    for g in range(2):
                        ps = psS.tile([128, 1024], F32, name="psd", tag="sc")
                        for kc in range(KC):
                            for i in range(2):
                                p = 2 * g + i
                                nc.tensor.matmul(
                                    ps[:, i * 512:(i + 1) * 512],
                                    lhsT=wq_sb[:, kc,
                                               p * 128:(p + 1) * 128],
                                    rhs=qchs[qq][:, kc, :],
                                    start=(kc == 0), stop=(kc == KC - 1))
                        for i in range(2):
                            p = 2 * g + i
                            nc.vector.tensor_scalar_add(
                                qhTs[p][:, sl],
                                ps[:, i * 512:(i + 1) * 512],
                                bq_sb[:, p:p + 1])

                ei = 0
                deferred = []
                for qhalf in range(2):
                    entries = [(p, hh, kt) for p in range(NP)
                               for hh in range(2) for kt in range(KT)]
                    pts, avs, av_pairs = {}, {}, {}
                    pending = []

                    def emit_av(si):
                        p, hh, kt = entries[si]
                        h = 2 * p + hh
                        if kt == 0:
                            avs[(p, hh)] = psAV.tile([128, 8, 128], F32,
                                                     name="av", tag="av")
                        av = avs[(p, hh)]
                        # av packs 4 slots per 2KB PSUM bank; start=True
                        # zeroes the WHOLE bank (zero region), so only the
                        # first slot per bank starts — later slots
                        # accumulate onto pending-zero bytes (read as zero).
                        for j in range(8):
                            nc.tensor.matmul(
                                av[:, j, 0:DV + 2],
                                lhsT=pts[si][:, j * 128:(j + 1) * 128],
                                rhs=vh_all[:, h, kt, :],
                                start=(kt == 0 and j % 4 == 0),
                                stop=(kt == KT - 1),
                                skip_group_check=True)
                        if kt != KT - 1:
                            return
                        # head complete: normalize (q on partitions)
                        if hh == 0:
                            av_pairs[p] = avsb_pool.tile(
                                [128, 8, 2, DV], F16, name="av_pair",
                                tag="avp")
                        rc = rc_pool.tile([128, 8], F32, name="rc", tag="rc")
                        nc.vector.reciprocal(rc, av[:, :, DV])
                        rc_bc = bass.AP(
                            tensor=rc.tensor, offset=rc.offset,
                            ap=list(rc[:, :].ap) + [[0, DV]])
                        nc.vector.scalar_tensor_tensor(
                            out=av_pairs[p][:, :, hh, :],
                            in0=av[:, :, 0:DV],
                            scalar=1.0, in1=rc_bc,
                            op0=AluOpType.mult, op1=AluOpType.mult)
                        if hh == 1:
                            for j in range(8):
                                nc.sync.dma_start_transpose(
                                    out=attnTs[p][:,
                                                  qhalf * 1024 + j * 128:
                                                  qhalf * 1024 +
                                                  (j + 1) * 128],
                                    in_=av_pairs[p][:, j, :, :])
                            # keep exp engines fed: spread the previous
                            # half's output projection between pairs
                            for _ in range(2):
                                if deferred:
                                    emit_oproj(deferred.pop(0))
                            if qhalf == 0 and p in (0, 1):
                                emit_qproj(p + 2)

                    for si, (p, hh, kt) in enumerate(entries):
                        hsl = slice(hh * 64, (hh + 1) * 64)
                        sc = psS.tile([128, 1024], F32, name="sc", tag="sc")
                        for half in range(2):
                            q0 = qhalf * 1024 + half * 512
                            nc.tensor.matmul(
                                sc[:, half * 512:(half + 1) * 512],
                                lhsT=khTs[p][hsl, kt * 128:(kt + 1) * 128],
                                rhs=qhTs[p][hsl, q0:q0 + 512],
                                start=True, stop=True)
                        pt = pt_pool.tile([128, 1024], F16, name="pt",
                                          tag="pt")
                        pts[si] = pt
                        on_pool = exp_on_pool(ei)
                        if on_pool:
                            scb = scsb_pool.tile([128, 1024], F32,
                                                 name="scb", tag="scb")
                            nc.vector.tensor_copy(scb, sc)
                            nc.gpsimd.tensor_tensor(pt, e_sb, scb,
                                                    AluOpType.pow)
                        else:
                            nc.scalar.activation(
                                pt, sc, mybir.ActivationFunctionType.Exp,
                                scale=0.125)
                        lag = POOL_LAG if on_pool else ACT_LAG
                        pending.append((si + lag, si))
                        ei += 1
                        while pending and pending[0][0] <= si:
                            emit_av(pending.pop(0)[1])
                    for _, psi in pending:
                        emit_av(psi)
                    pending = []
                    deferred = list(range(qhalf * 8, qhalf * 8 + 8))
                for t in deferred:
                    emit_oproj(t)

    nc.compile()
    return nc


_NC = None


def _get_nc():
    global _NC
    if _NC is None:
        _NC = build_nc()
    return _NC


def make_in_maps(inputs):
    f16 = np.float16
    q = np.asarray(inputs["q"], dtype=np.float32)
    k = np.asarray(inputs["k"], dtype=np.float32)
    v = np.asarray(inputs["v"], dtype=np.float32)
    Wq = np.asarray(inputs["Wq"], dtype=np.float32)
    Wk = np.asarray(inputs["Wk"], dtype=np.float32)
    Wv = np.asarray(inputs["Wv"], dtype=np.float32)
    Wo = np.asarray(inputs["Wo"], dtype=np.float32)
    bq = np.asarray(inputs["bq"], dtype=np.float32)
    bk = np.asarray(inputs["bk"], dtype=np.float32)
    bv = np.asarray(inputs["bv"], dtype=np.float32)

    def tile_xT(x):
        # x[b].T [1024, 2048] -> [qq 4, kc 8, 128, 512]
        xt = x.T.reshape(KC, 128, 4, 512)
        return np.ascontiguousarray(xt.transpose(2, 0, 1, 3)).astype(f16)

    def tile_w(W, sl):
        # [1024, 512] -> [128, kc 8, 512]
        return np.ascontiguousarray(
            W[:, sl].reshape(KC, 128, 512).transpose(1, 0, 2)).astype(f16)

    in_maps = []
    for c in range(8):
        b, g = divmod(c, 2)
        sl = slice(g * 512, (g + 1) * 512)
        in_maps.append({
            "qT": tile_xT(q[b]),
            "kT": tile_xT(k[b]),
            "vT": tile_xT(v[b]),
            "wq": tile_w(Wq, sl),
            "wk": tile_w(Wk, sl),
            "wv": tile_w(Wv, sl),
            "wo": np.ascontiguousarray(
                Wo[sl, :].reshape(NP, 128, 1024).transpose(1, 0, 2)
            ).astype(f16),
            "bq": np.ascontiguousarray(bq[sl].reshape(NP, 128).T),
            "bk": np.ascontiguousarray(bk[sl].reshape(NP, 128).T),
            "bv": np.ascontiguousarray(bv[sl]),
        })
    return in_maps


def gather_output(results, inputs):
    bo = np.asarray(inputs["bo"], dtype=np.float32)
    outs = [np.asarray(r["out"]) for r in results]
    full = np.stack([outs[2 * b] + outs[2 * b + 1] + bo for b in range(B)])
    return full.astype(np.float32)


def kernel(**inputs):
    nc = _get_nc()
    in_maps = make_in_maps(inputs)
    res = bass_utils.run_bass_kernel_spmd(nc, in_maps, core_ids=list(range(8)))
    return gather_output(res.results, inputs)


if __name__ == "__main__":
    build_nc()
    print("build OK")



# revision 82
# speedup vs baseline: 1.3455x; 1.3455x over previous
"""Multi-head attention (B=4, S=2048, d_model=1024, 16 heads x 64) on 8 trn2
NeuronCores — v3.

Sharding: core c -> (batch b = c//2, head-group g = c%2); 8 heads per core.
Host sums the two partial output projections per batch and adds bo.

v3 changes vs v2 (380.7us):
  - All projections use fp8e4 DoubleRow matmuls with host-side hi/lo
    splitting of x AND W (3 terms: xh*wh + xl*wh + xh*wl), contracting two
    128-element kc chunks per instruction at 0.5 cycles/row -> 4x fewer PE
    cycles than f16 with ~1e-3 accuracy.
  - Scores use single-term fp8 DoubleRow: qhT/khT are stored fp8 in a
    "quad" layout [128 part = 4 heads x 32 d, 2 d-halves, S] so one DR
    matmul contracts all 64 d dims from a 32-partition base (PE quadrant
    tile_position). Halves score PE cost; adds ~1.3e-2 rel error
    (gate is 2e-2).
  - AV + O-proj stay f16 (fp8 there fails the error budget).
  - PSUM: shared 3-deep [128,1024] ring (6 banks) for scores/proj, plus
    a 2-bank psAV pool for AV accumulators; oproj tiles borrow the psAV
    pool during its idle window at each sweep boundary.
  - AV emission lags scores by ~20 entries (tapering), with Pool-path
    entries permuted to the latest slots of each sweep; V-proj and Q/K
    tail chunks interleave into the stream so ACT/Pool start early.
  - Exp alternates ACT (native, from PSUM) / Pool (pow via DVE relay).
"""

import numpy as np
import ml_dtypes

import concourse.bass as bass
import concourse.bacc as bacc
import concourse.mybir as mybir
import concourse.tile as tile
from concourse import bass_utils
from concourse.alu_op_type import AluOpType

F32 = mybir.dt.float32
F16 = mybir.dt.float16
F8 = mybir.dt.float8e4
E4M3 = ml_dtypes.float8_e4m3
DR = mybir.MatmulPerfMode.DoubleRow

B, S, DM = 4, 2048, 1024
HPC = 8          # heads per core
DK = DV = 64
NP = HPC // 2    # head pairs per core = 4
KT = S // 128    # 16 k-tiles
KC = DM // 128   # 8 contraction chunks (4 DR pairs)
EXP8 = float(np.exp(0.125))
# Q/K/V weights (std ~0.02) sit below e4m3's normal range (min 2^-6), so
# the host scales them by W_SCALE before the fp8 hi/lo split; the
# projection drains multiply by 1/W_SCALE when writing SBUF.
W_SCALE = 64.0
# av group for score entry si is emitted at stream step si + 20 - kt//2:
# early kt lag 20, tail kt lag 13 — so a sweep's normalize has ~8 entries
# to drain before the next sweep's start=True reuses the psAV banks.
AV_LAG_HI, AV_LAG_TAPER = 20, 2

# exp engine split: ACT native Exp straight from PSUM; alternate entries
# go DVE-copy (PSUM->SBUF) + Pool pow (GPSIMD can't read PSUM).
_POOL_MOD = 8
_POOL_SET = (1, 3, 5, 7)   # alternate entries: even -> ACT, odd -> Pool
TAIL_TIGHT = {}
_AV_FIRST = False


def build_nc():
    nc = bacc.Bacc("TRN2", target_bir_lowering=False)

    xts = {}
    for nm in ("qTh", "qTl", "kTh", "kTl", "vTh", "vTl"):
        xts[nm] = nc.dram_tensor(nm, [4, KC, 128, 512], F8,
                                 kind="ExternalInput")
    wts = {}
    for nm in ("wqh", "wql", "wkh", "wkl", "wvh", "wvl"):
        wts[nm] = nc.dram_tensor(nm, [128, KC, 512], F8,
                                 kind="ExternalInput")
    wo = nc.dram_tensor("wo", [128, NP, 1024], F16, kind="ExternalInput")
    bqd = nc.dram_tensor("bqd", [128, 4], F32, kind="ExternalInput")
    bkd = nc.dram_tensor("bkd", [128, 4], F32, kind="ExternalInput")
    bvd = nc.dram_tensor("bvd", [512], F16, kind="ExternalInput")
    out = nc.dram_tensor("out", [S, 1024], F16, kind="ExternalOutput")

    def dram_ap(t, offset, ap):
        base = t[:]
        return bass.AP(tensor=base.tensor, offset=base.offset + offset, ap=ap)

    # [qq] chunk of an xT input as [128, kc, 512] (partition-major view)
    def x_chunk_ap(t, qq):
        return dram_ap(t, qq * KC * 128 * 512,
                       [[512, 128], [128 * 512, KC], [1, 512]])

    with tile.TileContext(nc) as tc:
        from contextlib import ExitStack

        with ExitStack() as est:
            # ---------------- persistent SBUF ----------------
            w_pool = est.enter_context(tc.tile_pool(name="wp", bufs=1))
            bias_pool = est.enter_context(tc.tile_pool(name="bias", bufs=1))
            vh_pool = est.enter_context(tc.tile_pool(name="vhp", bufs=1))
            qk_pool = est.enter_context(tc.tile_pool(name="qkt", bufs=1))
            at_pool = est.enter_context(tc.tile_pool(name="atp", bufs=1))
            e_pool = est.enter_context(tc.tile_pool(name="ep", bufs=1))
            x_pool = est.enter_context(tc.tile_pool(name="xch", bufs=1))
            pt_pool = est.enter_context(tc.tile_pool(name="ptp", bufs=26))
            scsb_pool = est.enter_context(tc.tile_pool(name="scsb", bufs=5))
            avsb_pool = est.enter_context(tc.tile_pool(name="avsb", bufs=2))
            rc_pool = est.enter_context(tc.tile_pool(name="rcp", bufs=2))
            fs_pool = est.enter_context(tc.tile_pool(name="osb", bufs=3))

            w_sb = {nm: w_pool.tile([128, KC, 512], F8, name=nm + "_sb")
                    for nm in ("wqh", "wql", "wkh", "wkl", "wvh", "wvl")}
            wo_sb = w_pool.tile([128, NP, 1024], F16, name="wo_sb")
            bq_sb = bias_pool.tile([128, 4], F32, name="bq_sb")
            bk_sb = bias_pool.tile([128, 4], F32, name="bk_sb")
            bv_bc = bias_pool.tile([128, 512], F16, name="bv_bc")
            e_sb = e_pool.tile([128, 8], F16, name="e_sb")
            nc.vector.memset(e_sb, EXP8)
            e_bc = bass.AP(tensor=e_sb.tensor, offset=e_sb.offset,
                           ap=[list(e_sb[:, :].ap[0]), [0, 1024]])

            # vh_all[k, h, t, 0:64] = vh; col 64 = 1.0 (denominator);
            # col 65 = zero pad (even fp16 moving element count)
            vh_all = vh_pool.tile([128, HPC, KT, DV + 2], F16, name="vh_all")
            nc.vector.memset(vh_all[:, :, :, DV:DV + 1], 1.0)
            nc.vector.memset(vh_all[:, :, :, DV + 1:DV + 2], 0.0)

            # fp8 quad layout: [128 part = 4 heads x 32 dims, 2 d-halves, S]
            qhTq, khTq = {}, {}
            for t in range(2):
                qhTq[t] = qk_pool.tile([128, 2, S], F8, name=f"qhTq{t}")
                khTq[t] = qk_pool.tile([128, 2, S], F8, name=f"khTq{t}")
            # split per qhalf so qh1 transposes don't serialize behind
            # qh0 oproj reads (tile-granular dependency tracking)
            attnTs = {(p, qh): at_pool.tile([128, 1024], F16,
                                            name=f"attnT{p}_{qh}")
                      for p in range(NP) for qh in range(2)}

            # ---------------- input DMA helpers ----------------
            xch = {}

            def load_chunk(kind, qq):
                t = x_pool.tile([128, KC, 512], F8, name=f"{kind}{qq}",
                                tag=f"x{kind}", bufs=2)
                xch[(kind, qq)] = t
                nc.sync.dma_start(out=t, in_=x_chunk_ap(xts[kind], qq))

            def load_w(nm):
                nc.sync.dma_start(out=w_sb[nm], in_=wts[nm][:, :, :])

            # warmup shortcut: tiny duplicate loads of the first half
            # (kc 0-3) of kTh chunk 0 + wkh, so the very first DR matmul
            # starts after ~2 small transfers instead of 2 full tiles
            kha = w_pool.tile([128, 2, 512], F8, name="kha")
            wha = w_pool.tile([128, 2, 512], F8, name="wha")
            nc.sync.dma_start(
                out=kha, in_=dram_ap(xts["kTh"], 0,
                                     [[512, 128], [128 * 512, 2], [1, 512]]))
            nc.sync.dma_start(out=wha, in_=wts["wkh"][:, 0:2, :])

            # prologue K/Q loads, term-major so hi*w_hi can start early
            load_chunk("kTh", 0)
            load_w("wkh")
            load_chunk("kTl", 0)
            load_w("wkl")
            load_chunk("qTh", 0)
            load_w("wqh")
            load_chunk("qTl", 0)
            load_w("wql")
            nc.sync.dma_start(out=bk_sb, in_=bkd[:, :])
            nc.sync.dma_start(out=bq_sb, in_=bqd[:, :])
            load_chunk("kTh", 1)
            load_chunk("kTl", 1)
            load_chunk("qTh", 1)
            load_chunk("qTl", 1)
            load_chunk("vTh", 0)
            load_w("wvh")
            load_chunk("vTl", 0)
            load_w("wvl")
            nc.sync.dma_start(out=wo_sb, in_=wo[:, :, :])
            bv_ap = bvd[:]
            nc.sync.dma_start(
                out=bv_bc,
                in_=bass.AP(tensor=bv_ap.tensor, offset=bv_ap.offset,
                            ap=[[0, 128]] + list(bv_ap.ap)),
            )
            for qq in (2, 3):
                load_chunk("kTh", qq)
                load_chunk("kTl", qq)
            load_chunk("vTh", 1)
            load_chunk("vTl", 1)
            load_chunk("vTh", 2)
            load_chunk("vTl", 2)
            load_chunk("vTh", 3)
            load_chunk("vTl", 3)
            for qq in (2, 3):
                load_chunk("qTh", qq)
                load_chunk("qTl", qq)

            # ---------------- PSUM pools ----------------
            # shared pool: scores / proj / oproj (short-lived, 2 banks ea)
            ps = est.enter_context(tc.tile_pool(name="ps", bufs=3,
                                                space="PSUM"))
            # av accumulators (live a whole 16-kt sweep -> own pool)
            psAV = est.enter_context(tc.tile_pool(name="psAV", bufs=1,
                                                  space="PSUM"))

            def ps_tile():
                return ps.tile([128, 1024], F32, name="pst", tag="ps")

            # ---- projection emitters (fp8 DR, 3 terms, 4 kc-pair steps) ----
            def proj_terms(kind, qq):
                h, l = kind + "h", kind + "l"  # noqa: E741
                wh, wl = "w" + kind[0].lower() + "h", "w" + kind[0].lower() + "l"
                return ((xch[(h, qq)], w_sb[wh]), (xch[(l, qq)], w_sb[wh]),
                        (xch[(h, qq)], w_sb[wl]))

            def emit_qk_projpair(kind, qq, tile_idx, dest, bias, warm=None,
                                 from_av=False, drain_act=False):
                """One ps tile covering W-col blocks (2*tile_idx,
                2*tile_idx+1) for chunk qq of Q or K projection."""
                if from_av:
                    # mid-stream deferred Q chunks borrow the psAV pool's
                    # sweep-boundary idle window instead of the score ring
                    pst = psAV.tile([128, HPC, 128], F32, name="pq",
                                    tag="av").rearrange("p a b -> p (a b)")
                else:
                    pst = ps_tile()
                terms = proj_terms(kind, qq)
                for ti, (xt, wt) in enumerate(terms):
                    for st in range(4):
                        xs, ws = xt, wt
                        if warm is not None and ti == 0 and st < 1:
                            xs, ws = warm
                        for blk in range(2):
                            gb = 2 * tile_idx + blk
                            nc.tensor.matmul(
                                pst[:, blk * 512:(blk + 1) * 512],
                                lhsT=ws[:, 2 * st:2 * st + 2,
                                        gb * 128:(gb + 1) * 128],
                                rhs=xs[:, 2 * st:2 * st + 2, :],
                                start=(ti == 0 and st == 0),
                                stop=(ti == 2 and st == 3),
                                perf_mode=DR)
                if drain_act:
                    # ACT is idle during the prologue: drain there via
                    # Identity (shares the exp act-table, so no reload)
                    for blk in range(2):
                        gb = 2 * tile_idx + blk
                        nc.scalar.activation(
                            dest[tile_idx][:, blk, qq * 512:(qq + 1) * 512],
                            pst[:, blk * 512:(blk + 1) * 512],
                            mybir.ActivationFunctionType.Identity,
                            bias=bias[:, gb:gb + 1],
                            scale=1.0 / W_SCALE)
                    return
                # drain both blocks in one DVE op: blocks (2t, 2t+1) are
                # quad t halves 0,1 -> dest[t][:, 0:2, qq*512:...]
                bias_bc = bass.AP(
                    tensor=bias.tensor, offset=bias.offset + 2 * tile_idx,
                    ap=[[4, 128], [1, 2], [0, 512]])
                nc.vector.scalar_tensor_tensor(
                    out=dest[tile_idx][:, :, qq * 512:(qq + 1) * 512],
                    in0=pst.rearrange("p (b f) -> p b f", b=2),
                    scalar=1.0 / W_SCALE, in1=bias_bc,
                    op0=AluOpType.mult, op1=AluOpType.add)

            vps = {}

            def emit_v_tile(st):
                """V projection for seq tile st -> vh_all[:, :, st, 0:64]."""
                sq, j = divmod(st, 4)
                if st % 2 == 0:
                    vps[st // 2] = ps_tile()
                pst = vps[st // 2]
                half = st % 2
                dst = pst[:, half * 512:(half + 1) * 512]
                for ti, (xt, wt) in enumerate(proj_terms("vT", sq)):
                    for stp in range(4):
                        nc.tensor.matmul(
                            dst,
                            lhsT=xt[:, 2 * stp:2 * stp + 2,
                                    j * 128:(j + 1) * 128],
                            rhs=wt[:, 2 * stp:2 * stp + 2, :],
                            start=(ti == 0 and stp == 0),
                            stop=(ti == 2 and stp == 3),
                            perf_mode=DR)
                nc.vector.scalar_tensor_tensor(
                    out=vh_all[:, :, st, 0:DV],
                    in0=dst.rearrange("p (h d) -> p h d", h=HPC),
                    scalar=1.0 / W_SCALE,
                    in1=bv_bc.rearrange("p (h d) -> p h d", h=HPC),
                    op0=AluOpType.mult, op1=AluOpType.add)

            # ---------------- prologue projections ----------------
            # chunk-major (both quad tiles per chunk — the 2-deep x-buffer
            # ring would deadlock otherwise); entry-0's needs (K c0 t0,
            # Q c0/c1 t0) come first within that constraint.
            emit_qk_projpair("kT", 0, 0, khTq, bk_sb, warm=(kha, wha))
            emit_qk_projpair("kT", 0, 1, khTq, bk_sb, drain_act=True)
            emit_qk_projpair("qT", 0, 0, qhTq, bq_sb)
            emit_qk_projpair("qT", 0, 1, qhTq, bq_sb, drain_act=True)
            emit_qk_projpair("qT", 1, 0, qhTq, bq_sb)
            emit_qk_projpair("qT", 1, 1, qhTq, bq_sb, drain_act=True)
            for qq in (1, 2, 3):
                emit_qk_projpair("kT", qq, 0, khTq, bk_sb)
                emit_qk_projpair("kT", qq, 1, khTq, bk_sb, drain_act=True)

            # ---------------- attention stream ----------------
            entries = [(qh, p, hh, kt) for qh in range(2) for p in range(NP)
                       for hh in range(2) for kt in range(KT)]
            NE = len(entries)  # 256
            pts, avs, av_pairs = {}, {}, {}
            deferred = []
            pend_dma, pend_store = [], []

            cur_step = [0]

            def flush_dmas(min_age):
                while pend_dma and cur_step[0] - pend_dma[0][0] >= min_age:
                    _, qh2, p2 = pend_dma.pop(0)
                    for j in range(8):
                        nc.sync.dma_start_transpose(
                            out=attnTs[(p2, qh2)][:, j * 128:(j + 1) * 128],
                            in_=av_pairs[(qh2, p2)][:, j, :, :])
                while pend_store and cur_step[0] - pend_store[0][0] \
                        >= min_age:
                    _, tsl2, fs2 = pend_store.pop(0)
                    nc.sync.dma_start(out=out[tsl2, :], in_=fs2)

            def emit_av(si):
                qh, p, hh, kt = entries[si]
                h = 2 * p + hh
                if kt == 0:
                    avs[(qh, p, hh)] = psAV.tile([128, HPC, 128], F32,
                                                 name="av", tag="av")
                av = avs[(qh, p, hh)]
                # av packs 4 slots per 2KB PSUM bank; start=True zeroes the
                # WHOLE bank, so only the first slot per bank starts.
                for j in range(8):
                    nc.tensor.matmul(
                        av[:, j, 0:DV + 2],
                        lhsT=pts[si][:, j * 128:(j + 1) * 128],
                        rhs=vh_all[:, h, kt, :],
                        start=(kt == 0 and j % 4 == 0),
                        stop=(kt == KT - 1),
                        skip_group_check=True)
                if kt != KT - 1:
                    return
                # head complete: normalize (q on partitions)
                if hh == 0:
                    av_pairs[(qh, p)] = avsb_pool.tile(
                        [128, 8, 2, DV], F16, name="av_pair", tag="avp")
                rc = rc_pool.tile([128, 8], F32, name="rc", tag="rc")
                nc.vector.reciprocal(rc, av[:, :, DV])
                rc_bc = bass.AP(
                    tensor=rc.tensor, offset=rc.offset,
                    ap=list(rc[:, :].ap) + [[0, DV]])
                nc.vector.scalar_tensor_tensor(
                    out=av_pairs[(qh, p)][:, :, hh, :],
                    in0=av[:, :, 0:DV],
                    scalar=1.0, in1=rc_bc,
                    op0=AluOpType.mult, op1=AluOpType.mult)
                if hh == 1:
                    # defer the transposes ~2 steps so their normalize
                    # dependency is already satisfied when they reach the
                    # head of the SP queue (a waiting DMA holds SP.SEQ)
                    pend_dma.append((cur_step[0], qh, p))
                    if p == NP - 1:
                        deferred.extend(range(qh * 8, qh * 8 + 8))

            def emit_oproj(t, from_ring=False):
                tqh, tloc = divmod(t, 8)
                tsl = slice(t * 128, (t + 1) * 128)
                lsl = slice(tloc * 128, (tloc + 1) * 128)
                # oproj PSUM comes from the psAV pool (free for ~8 steps at
                # every sweep boundary) — never stalls the score ring. In
                # the tail (stream over) the main ring is free: use it.
                if from_ring:
                    pf = ps_tile()
                else:
                    pf = psAV.tile([128, HPC, 128], F32, name="pf",
                                   tag="av").rearrange("p a b -> p (a b)")
                for c in range(NP):
                    for half in range(2):
                        nc.tensor.matmul(
                            pf[:, half * 512:(half + 1) * 512],
                            lhsT=attnTs[(c, tqh)][:, lsl],
                            rhs=wo_sb[:, c, half * 512:(half + 1) * 512],
                            start=(c == 0), stop=(c == NP - 1))
                fs = fs_pool.tile([128, 1024], F16, name="fs", tag="fs")
                # split the drain across ACT+DVE so the boundary ACT hit
                # (which delays the next sweep's exps) is halved
                nc.scalar.copy(fs[:, 0:512], pf[:, 0:512])
                nc.vector.tensor_copy(fs[:, 512:1024], pf[:, 512:1024])
                pend_store.append((cur_step[0], tsl, fs))

            # interleave schedules (entry index -> work)
            q23_at = {61: ("qT", 2, 0), 77: ("qT", 2, 1),
                      93: ("qT", 3, 0), 109: ("qT", 3, 1)}
            # av emission steps: per sweep the slot set is
            # {si + 20 - kt//2}, but kts are PERMUTED into slots so that
            # Pool-path entries (slow relay+pow chain) land in the latest
            # slots. kt0 keeps the first slot (start=True must run first),
            # kt15 the last (stop=True must run last).
            def on_pool(si):
                kt = entries[si][3]
                return (si % _POOL_MOD in _POOL_SET and 3 <= kt <= 14
                        and (si // 16) not in TAIL_TIGHT)

            av_at = {}
            for s in range(16):
                slots = sorted(
                    16 * s + kt + AV_LAG_HI - kt // AV_LAG_TAPER
                    - TAIL_TIGHT.get(s, 0) for kt in range(KT))
                mid = [kt for kt in range(1, 15) if not on_pool(16 * s + kt)]
                mid += [kt for kt in range(1, 15) if on_pool(16 * s + kt)]
                order = [0] + mid + [15]
                for m, kta in enumerate(order):
                    av_at.setdefault(slots[m], []).append(16 * s + kta)
            last_step = max(av_at)
            for i in range(last_step + 1):
                cur_step[0] = i
                flush_dmas(2)
                # av groups emitted AHEAD of the score when K_AV_FIRST=1
                # (the PE is in-order; runnable work before a potentially
                # ring-stalled score fills the stall).
                if _AV_FIRST:
                    for si in av_at.get(i, ()):
                        if not on_pool(si):
                            emit_av(si)
                if i < NE:
                    qh, p, hh, kt = entries[i]
                    h = 2 * p + hh
                    t, j = divmod(h, 4)
                    sc = ps_tile()
                    for half in range(2):
                        q0 = qh * 1024 + half * 512
                        nc.tensor.matmul(
                            sc[:, half * 512:(half + 1) * 512],
                            lhsT=khTq[t][32 * j:32 * j + 32, :,
                                         kt * 128:(kt + 1) * 128],
                            rhs=qhTq[t][32 * j:32 * j + 32, :, q0:q0 + 512],
                            start=True, stop=True, perf_mode=DR,
                            tile_position=(32 * j, 0))
                    pt = pt_pool.tile([128, 1024], F16, name="pt", tag="pt")
                    pts[i] = pt
                    if on_pool(i):
                        scb = scsb_pool.tile([128, 1024], F16, name="scb",
                                             tag="scb")
                        nc.vector.tensor_copy(scb, sc)
                        nc.gpsimd.tensor_tensor(pt, e_bc, scb, AluOpType.pow)
                    else:
                        nc.scalar.activation(
                            pt, sc, mybir.ActivationFunctionType.Exp,
                            scale=0.125)
                    # fixed interleaves keyed on the score entry index
                    if i < 16:
                        emit_v_tile(i)
                    if i in q23_at:
                        kind, qq, ti2 = q23_at[i]
                        emit_qk_projpair(kind, qq, ti2, qhTq, bq_sb,
                                         from_av=True)
                for si in av_at.get(i, ()):
                    if not _AV_FIRST or on_pool(si):
                        emit_av(si)
                # pop one oproj per sweep boundary (psAV free window)
                if deferred and i % 16 == 15 and i >= 16:
                    emit_oproj(deferred.pop(0))
                cur_step[0] = i + 1
            flush_dmas(0)
            for n2, t in enumerate(deferred):
                flush_dmas(0)
                # alternate pools in the tail: psAV frees instantly after
                # the last sweep; the ring frees as the last exps drain
                emit_oproj(t, from_ring=(n2 % 4 != 0))
            cur_step[0] += 16
            flush_dmas(0)

    nc.compile()
    return nc


_NC = None


def _get_nc():
    global _NC
    if _NC is None:
        _NC = build_nc()
    return _NC


def _split8(x):
    hi = x.astype(E4M3)
    lo = (x - hi.astype(np.float32)).astype(E4M3)
    return hi, lo


def _qk_col_perm():
    """Within a core's 512 Q/K projection columns: block b = (quad t,
    d-half): cols (j*32 + r) -> head 4t+j, d = 32*half + r."""
    perm = np.zeros(512, dtype=np.int64)
    for b in range(4):
        t, half = divmod(b, 2)
        for j in range(4):
            for r in range(32):
                perm[b * 128 + j * 32 + r] = (4 * t + j) * 64 + 32 * half + r
    return perm


_PERM = _qk_col_perm()


def make_in_maps(inputs):
    q = np.asarray(inputs["q"], dtype=np.float32)
    k = np.asarray(inputs["k"], dtype=np.float32)
    v = np.asarray(inputs["v"], dtype=np.float32)
    Wq = np.asarray(inputs["Wq"], dtype=np.float32)
    Wk = np.asarray(inputs["Wk"], dtype=np.float32)
    Wv = np.asarray(inputs["Wv"], dtype=np.float32)
    Wo = np.asarray(inputs["Wo"], dtype=np.float32)
    bq = np.asarray(inputs["bq"], dtype=np.float32)
    bk = np.asarray(inputs["bk"], dtype=np.float32)
    bv = np.asarray(inputs["bv"], dtype=np.float32)

    def tile_xT(x):
        # x[b].T [1024, 2048] -> [qq 4, kc 8, 128, 512]
        xt = x.T.reshape(KC, 128, 4, 512)
        return np.ascontiguousarray(xt.transpose(2, 0, 1, 3))

    def tile_w(W):
        # [1024, 512] -> [128, kc 8, 512]
        return np.ascontiguousarray(
            W.reshape(KC, 128, 512).transpose(1, 0, 2))

    # hi/lo splits of x are shared across the two head-group cores per batch
    xsplits = {}
    for nm, x in (("qT", q), ("kT", k), ("vT", v)):
        for b in range(B):
            xsplits[(nm, b)] = _split8(tile_xT(x[b]))

    in_maps = []
    for c in range(8):
        b, g = divmod(c, 2)
        sl = slice(g * 512, (g + 1) * 512)
        m = {}
        for nm in ("qT", "kT", "vT"):
            hi, lo = xsplits[(nm, b)]
            m[nm + "h"], m[nm + "l"] = hi, lo
        for nm, W, perm in (("wq", Wq, _PERM), ("wk", Wk, _PERM),
                            ("wv", Wv, None)):
            Wsl = W[:, sl] * W_SCALE
            if perm is not None:
                Wsl = Wsl[:, perm]
            hi, lo = _split8(Wsl.reshape(-1))
            m[nm + "h"] = tile_w(hi.astype(np.float32).reshape(1024, 512)
                                 ).astype(E4M3)
            m[nm + "l"] = tile_w(lo.astype(np.float32).reshape(1024, 512)
                                 ).astype(E4M3)
        m["wo"] = np.ascontiguousarray(
            Wo[sl, :].reshape(NP, 128, 1024).transpose(1, 0, 2)
        ).astype(np.float16)
        m["bqd"] = np.ascontiguousarray(
            bq[sl][_PERM].reshape(4, 128).T).astype(np.float32)
        m["bkd"] = np.ascontiguousarray(
            bk[sl][_PERM].reshape(4, 128).T).astype(np.float32)
        m["bvd"] = np.ascontiguousarray(bv[sl]).astype(np.float16)
        in_maps.append(m)
    return in_maps


def gather_output(results, inputs):
    bo = np.asarray(inputs["bo"], dtype=np.float32)
    outs = [np.asarray(r["out"]).astype(np.float32) for r in results]
    full = np.stack([outs[2 * b] + outs[2 * b + 1] + bo for b in range(B)])
    return full.astype(np.float32)


def kernel(**inputs):
    nc = _get_nc()
    in_maps = make_in_maps(inputs)
    res = bass_utils.run_bass_kernel_spmd(nc, in_maps, core_ids=list(range(8)))
    return gather_output(res.results, inputs)


if __name__ == "__main__":
    build_nc()
    print("build OK")
